# revision 22
# baseline (speedup 1.0000x reference)
"""ConcatRelationModule Bass kernel for 8 trn2 NeuronCores — v6.

Per edge e in [0, 16383):
    x      = concat(inputs[heads[e], 0, :], inputs[e + 1, 1, :])     # [512]
    h      = tanh(concat(x @ W_FOH, x @ W_FOM) + b1)                 # [1024]
    h2     = tanh(h @ W2 + b2)                                       # [256]
    out[e] = h2 @ W3 + b3                                            # [E, 64]

v17 = the proven v1 pipeline (per-group gathers -> transposes -> L1 ->
L2 -> L3, loads spread across the kernel so the per-core HBM share
(~90GB/s with all 8 cores bursting in lockstep) is never exceeded)
with surgical changes:
  - modifier (bwd) half of x host-pretransposed and DMAd per group in
    feature-major form: half the PE transposes and DVE copies are gone,
    and each group's L1 starts on the modifier half before its gather
    subtiles have landed (kc order 2,3,0,1)
  - PE warm-up matmuls on scratch during the DMA prologue so the HAM
    clock gate (1.2 -> 2.4 GHz) flips before real matmuls start
  - b1/b2 packed into one f32 load; b3 added on host; L3 output copied
    PSUM->SBUF on Vector instead of a Scalar ACT (shorter tail)
"""

import os

import numpy as np
import ml_dtypes

import concourse.bass as bass
import concourse.bacc as bacc
import concourse.mybir as mybir
import concourse.tile as tile
from concourse.bass import IndirectOffsetOnAxis
from concourse.bass_utils import run_bass_kernel_spmd
from concourse.masks import make_identity

N_TOKENS = 16384
LD = 256
HID = 512
HID2 = 256
NREL = 64
NCORES = 8
E = N_TOKENS - 1
EPC = N_TOKENS // NCORES  # 2048
P = 128
SUB = EPC // P            # 16
N_WARMUP = 8

GROUPS = [(0, 256), (256, 512), (768, 512), (1280, 512), (1792, 256)]

LAST_RESULTS = None
_CACHE = {}


def _build():
    bf16 = mybir.dt.bfloat16
    f32 = mybir.dt.float32

    nc = bacc.Bacc()
    fwd = nc.declare_dram_parameter("fwd", [N_TOKENS, LD], bf16, isOutput=False)
    bwdT = nc.declare_dram_parameter("bwdT", [P, 2, EPC], bf16, isOutput=False)
    headsT = nc.declare_dram_parameter(
        "headsT", [P, SUB], mybir.dt.int32, isOutput=False)
    w1 = nc.declare_dram_parameter("w1", [2 * LD, 2 * HID], bf16, isOutput=False)
    w2p = nc.declare_dram_parameter("w2p", [P, 8, HID2], bf16, isOutput=False)
    w3p = nc.declare_dram_parameter("w3p", [P, 2, NREL], bf16, isOutput=False)
    bpack = nc.declare_dram_parameter("bpack", [P, 10], f32, isOutput=False)
    outT = nc.declare_dram_parameter("outT", [NREL, EPC], f32, isOutput=True)

    Tanh = mybir.ActivationFunctionType.Tanh

    with tile.TileContext(nc) as tc:
        with (
            tc.tile_pool(name="const", bufs=1) as const_pool,
            tc.tile_pool(name="xh", bufs=3) as xh_pool,
            tc.tile_pool(name="xm", bufs=3) as xm_pool,
            tc.tile_pool(name="xT", bufs=3) as xT_pool,
            tc.tile_pool(name="h1", bufs=9) as h1_pool,
            tc.tile_pool(name="h2", bufs=3) as h2_pool,
            tc.tile_pool(name="outs", bufs=2) as out_pool,
            tc.tile_pool(name="pt", bufs=3, space="PSUM") as pt_pool,
            tc.tile_pool(name="ph", bufs=3, space="PSUM") as ph_pool,
            tc.tile_pool(name="pj", bufs=2, space="PSUM") as pj_pool,
        ):
            # headsT via gpsimd's own queue: the gathers follow on the same
            # engine, so no cross-engine semaphore hop gates them
            hT_sb = const_pool.tile([P, SUB], mybir.dt.int32)
            nc.gpsimd.dma_start(hT_sb[:], headsT[:])

            # gpsimd: scratch memset (warm-up data), identity, then gathers
            warm_sb = const_pool.tile([P, 512], bf16)
            nc.gpsimd.memset(warm_sb[:], 0)
            ident = const_pool.tile([P, P], bf16)
            make_identity(nc, ident[:])

            xg_tiles = [None] * len(GROUPS)
            xm_tiles = [None] * len(GROUPS)

            def load_group(gi):
                start, size = GROUPS[gi]
                ns = size // P
                xh = xh_pool.tile([P, ns, LD], bf16, tag="xh", name=f"xh_{gi}")
                for s in range(ns):
                    t = start // P + s
                    nc.gpsimd.indirect_dma_start(
                        out=xh[:, s, :],
                        out_offset=None,
                        in_=fwd[:],
                        in_offset=IndirectOffsetOnAxis(ap=hT_sb[:, t:t + 1], axis=0),
                    )
                # pre-transposed modifier half: direct, feature-major
                xm = xm_pool.tile([P, 2, size], bf16, tag="xm", name=f"xm_{gi}")
                nc.sync.dma_start(xm[:], bwdT[:, :, start:start + size])
                xg_tiles[gi] = xh
                xm_tiles[gi] = xm

            # PE warm-up on scratch; output never read.  Queued before any
            # real matmul so the clock gate flips during the DMA prologue.
            wps = pt_pool.tile([P, 512], f32, tag="pt", name="warmup")
            for i in range(N_WARMUP):
                nc.tensor.matmul(
                    out=wps[:], lhsT=warm_sb[:, 0:P], rhs=warm_sb[:],
                    start=True, stop=True,
                )

            load_group(0)
            load_group(1)
            # w1 split per k-chunk; modifier chunks (rows 256..511) first
            w1_sb = [const_pool.tile([P, 2 * HID], bf16, tag=f"w1_{kc}",
                                     name=f"w1_{kc}")
                     for kc in range(4)]
            for kc in (2, 3):
                nc.sync.dma_start(w1_sb[kc][:], w1[kc * P:(kc + 1) * P, :])
            bp_sb = const_pool.tile([P, 10], f32)
            nc.sync.dma_start(bp_sb[:], bpack[:])
            load_group(2)
            for kc in (0, 1):
                nc.sync.dma_start(w1_sb[kc][:], w1[kc * P:(kc + 1) * P, :])
            w2_sb = const_pool.tile([P, 8, HID2], bf16)
            nc.sync.dma_start(w2_sb[:], w2p[:])
            load_group(3)
            w3_sb = const_pool.tile([P, 2, NREL], bf16)
            nc.sync.dma_start(w3_sb[:], w3p[:])
            load_group(4)

            xT_tiles = [None] * len(GROUPS)

            def emit_transpose(gi):
                start, size = GROUPS[gi]
                xh = xg_tiles[gi]
                xTs = []
                for kc in range(2):  # head half only
                    col = kc * P
                    pt = pt_pool.tile([P, size], bf16, tag="pt",
                                      name=f"pt_{gi}_{kc}")
                    for s in range(size // P):
                        nc.tensor.transpose(
                            pt[:, s * P:(s + 1) * P],
                            xh[:, s, col:col + P], ident[:])
                    xT = xT_pool.tile([P, size], bf16, tag="xT",
                                      name=f"xT_{gi}_{kc}")
                    nc.vector.tensor_copy(out=xT[:], in_=pt[:])
                    xTs.append(xT)
                xT_tiles[gi] = xTs

            emit_transpose(0)
            for gi, (start, size) in enumerate(GROUPS):
                xTs = xT_tiles[gi]
                xm = xm_tiles[gi]
                # ---- L1: modifier half first (kc 2,3), then head (0,1) ----
                h1s = []
                for hc in range(8):
                    ph = ph_pool.tile([P, size], f32, tag="ph",
                                      name=f"ph_{gi}_{hc}")
                    for i, kc in enumerate((2, 3, 0, 1)):
                        rhs = xm[:, kc - 2, :] if kc >= 2 else xTs[kc][:]
                        nc.tensor.matmul(
                            out=ph[:],
                            lhsT=w1_sb[kc][:, hc * P:(hc + 1) * P],
                            rhs=rhs,
                            start=(i == 0),
                            stop=(i == 3),
                        )
                    h1 = h1_pool.tile([P, size], bf16, tag="h1",
                                      name=f"h1_{gi}_{hc}")
                    nc.scalar.activation(
                        out=h1[:], in_=ph[:], func=Tanh,
                        bias=bp_sb[:, hc:hc + 1],
                    )
                    h1s.append(h1)

                # transpose the NEXT group while this group's L2/L3 run
                if gi + 1 < len(GROUPS):
                    emit_transpose(gi + 1)

                # ---- L2 ----
                h2s = []
                for jc in range(2):
                    pj = pj_pool.tile([P, size], f32, tag="pj",
                                      name=f"pj_{gi}_{jc}")
                    for kc in range(8):
                        nc.tensor.matmul(
                            out=pj[:],
                            lhsT=w2_sb[:, kc, jc * P:(jc + 1) * P],
                            rhs=h1s[kc][:],
                            start=(kc == 0),
                            stop=(kc == 7),
                        )
                    h2 = h2_pool.tile([P, size], bf16, tag="h2",
                                      name=f"h2_{gi}_{jc}")
                    nc.scalar.activation(
                        out=h2[:], in_=pj[:], func=Tanh,
                        bias=bp_sb[:, 8 + jc:9 + jc],
                    )
                    h2s.append(h2)

                # ---- L3 (b3 on host) ----
                po = pt_pool.tile([NREL, size], f32, tag="pt", name=f"po_{gi}")
                for kc in range(2):
                    nc.tensor.matmul(
                        out=po[:],
                        lhsT=w3_sb[:, kc, :],
                        rhs=h2s[kc][:],
                        start=(kc == 0),
                        stop=(kc == 1),
                    )
                o = out_pool.tile([NREL, size], f32, tag="o", name=f"o_{gi}")
                nc.vector.tensor_copy(out=o[:], in_=po[:])
                nc.sync.dma_start(outT[:, start:start + size], o[:])

    nc.finalize()
    return nc


def _prep_inputs(inputs, rhidLayerFOH, rhidLayerFOM, rcatBias, rhid2Layer,
                 rhid2Bias, routLayer, routBias, heads):
    wdt = ml_dtypes.bfloat16
    inputs = np.asarray(inputs, dtype=np.float32)
    heads = np.asarray(heads)

    fwd = np.ascontiguousarray(inputs[:, 0, :]).astype(wdt)
    bwd_full = inputs[:, 1, :]
    mods_pad = np.concatenate(
        [np.arange(1, N_TOKENS), [N_TOKENS - 1]]).astype(np.int64)
    heads_pad = np.concatenate([heads.astype(np.int64), [0]]).astype(np.int32)

    w1 = np.ascontiguousarray(
        np.concatenate([np.asarray(rhidLayerFOH), np.asarray(rhidLayerFOM)],
                       axis=1)).astype(wdt)                      # [512, 1024]
    w2p = np.ascontiguousarray(
        np.asarray(rhid2Layer, dtype=np.float32)
        .reshape(8, P, HID2).transpose(1, 0, 2)).astype(wdt)
    w3p = np.ascontiguousarray(
        np.asarray(routLayer, dtype=np.float32)
        .reshape(2, P, NREL).transpose(1, 0, 2)).astype(wdt)
    b1 = np.asarray(rcatBias, dtype=np.float32).reshape(8, P).T
    b2 = np.asarray(rhid2Bias, dtype=np.float32).reshape(2, P).T
    bpack = np.ascontiguousarray(np.concatenate([b1, b2], axis=1))

    in_maps = []
    for c in range(NCORES):
        sl = slice(c * EPC, (c + 1) * EPC)
        bwd_c = bwd_full[mods_pad[sl]]                           # [2048, 256]
        bwdT_c = np.ascontiguousarray(
            bwd_c.T.reshape(2, P, EPC).transpose(1, 0, 2)).astype(wdt)
        headsT_c = np.ascontiguousarray(heads_pad[sl].reshape(SUB, P).T)
        in_maps.append({
            "fwd": fwd, "bwdT": bwdT_c, "headsT": headsT_c,
            "w1": w1, "w2p": w2p, "w3p": w3p, "bpack": bpack,
        })
    return in_maps


def kernel(inputs, rhidLayerFOH, rhidLayerFOM, rcatBias, rhid2Layer, rhid2Bias,
           routLayer, routBias, heads):
    global LAST_RESULTS

    in_maps = _prep_inputs(inputs, rhidLayerFOH, rhidLayerFOM, rcatBias,
                           rhid2Layer, rhid2Bias, routLayer, routBias, heads)

    if "nc" not in _CACHE:
        _CACHE["nc"] = _build()
    nc = _CACHE["nc"]

    trace_dir = os.environ.get("KERNEL_TRACE_DIR") or None
    res = run_bass_kernel_spmd(nc, in_maps, list(range(NCORES)), tmpdir=trace_dir)
    LAST_RESULTS = res

    outT = np.concatenate([r["outT"] for r in res.results], axis=1)
    out = outT.T[:E] + np.asarray(routBias, dtype=np.float32)[None, :]
    return np.ascontiguousarray(out).astype(np.float32)


# revision 24
# speedup vs baseline: 1.0274x; 1.0274x over previous
"""ConcatRelationModule Bass kernel for 8 trn2 NeuronCores — v6.

Per edge e in [0, 16383):
    x      = concat(inputs[heads[e], 0, :], inputs[e + 1, 1, :])     # [512]
    h      = tanh(concat(x @ W_FOH, x @ W_FOM) + b1)                 # [1024]
    h2     = tanh(h @ W2 + b2)                                       # [256]
    out[e] = h2 @ W3 + b3                                            # [E, 64]

v18 = the proven v1 pipeline (per-group gathers -> transposes -> L1 ->
L2 -> L3, loads spread across the kernel so the per-core HBM share
(~90GB/s with all 8 cores bursting in lockstep) is never exceeded)
with surgical changes:
  - modifier (bwd) half of x host-pretransposed and DMAd per group in
    feature-major form: half the PE transposes and DVE copies are gone,
    and each group's L1 starts on the modifier half before its gather
    subtiles have landed (kc order 2,3,0,1)
  - PE warm-up matmuls on scratch during the DMA prologue so the HAM
    clock gate (1.2 -> 2.4 GHz) flips before real matmuls start
  - b1/b2 packed into one f32 load; b3 added on host; L3 output copied
    PSUM->SBUF on Vector instead of a Scalar ACT (shorter tail)
"""

import os

import numpy as np
import ml_dtypes

import concourse.bass as bass
import concourse.bacc as bacc
import concourse.mybir as mybir
import concourse.tile as tile
from concourse.bass import IndirectOffsetOnAxis
from concourse.bass_utils import run_bass_kernel_spmd
from concourse.masks import make_identity

N_TOKENS = 16384
LD = 256
HID = 512
HID2 = 256
NREL = 64
NCORES = 8
E = N_TOKENS - 1
EPC = N_TOKENS // NCORES  # 2048
P = 128
SUB = EPC // P            # 16
N_WARMUP = 8

GROUPS = [(0, 128), (128, 128), (256, 512), (768, 512), (1280, 512),
          (1792, 256)]

LAST_RESULTS = None
_CACHE = {}


def _build():
    bf16 = mybir.dt.bfloat16
    f32 = mybir.dt.float32

    nc = bacc.Bacc()
    fwd = nc.declare_dram_parameter("fwd", [N_TOKENS, LD], bf16, isOutput=False)
    bwdT = nc.declare_dram_parameter("bwdT", [P, 2, EPC], bf16, isOutput=False)
    headsT = nc.declare_dram_parameter(
        "headsT", [P, SUB], mybir.dt.int32, isOutput=False)
    w1 = nc.declare_dram_parameter("w1", [2 * LD, 2 * HID], bf16, isOutput=False)
    w2p = nc.declare_dram_parameter("w2p", [P, 8, HID2], bf16, isOutput=False)
    w3p = nc.declare_dram_parameter("w3p", [P, 2, NREL], bf16, isOutput=False)
    bpack = nc.declare_dram_parameter("bpack", [P, 10], f32, isOutput=False)
    outT = nc.declare_dram_parameter("outT", [NREL, EPC], f32, isOutput=True)

    Tanh = mybir.ActivationFunctionType.Tanh

    with tile.TileContext(nc) as tc:
        with (
            tc.tile_pool(name="const", bufs=1) as const_pool,
            tc.tile_pool(name="xh", bufs=3) as xh_pool,
            tc.tile_pool(name="xm", bufs=3) as xm_pool,
            tc.tile_pool(name="xT", bufs=3) as xT_pool,
            tc.tile_pool(name="h1", bufs=9) as h1_pool,
            tc.tile_pool(name="h2", bufs=3) as h2_pool,
            tc.tile_pool(name="outs", bufs=2) as out_pool,
            tc.tile_pool(name="pt", bufs=3, space="PSUM") as pt_pool,
            tc.tile_pool(name="ph", bufs=3, space="PSUM") as ph_pool,
            tc.tile_pool(name="pj", bufs=2, space="PSUM") as pj_pool,
        ):
            # headsT via gpsimd's own queue: the gathers follow on the same
            # engine, so no cross-engine semaphore hop gates them
            hT_sb = const_pool.tile([P, SUB], mybir.dt.int32)
            nc.gpsimd.dma_start(hT_sb[:], headsT[:])

            # gpsimd: scratch memset (warm-up data), identity, then gathers
            warm_sb = const_pool.tile([P, 512], bf16)
            nc.gpsimd.memset(warm_sb[:], 0)
            bp_sb = const_pool.tile([P, 10], f32)
            nc.sync.dma_start(bp_sb[:], bpack[:])
            ident = const_pool.tile([P, P], bf16)

            xg_tiles = [None] * len(GROUPS)
            xm_tiles = [None] * len(GROUPS)

            def load_group(gi):
                start, size = GROUPS[gi]
                ns = size // P
                xh = xh_pool.tile([P, ns, LD], bf16, tag="xh", name=f"xh_{gi}")
                for s in range(ns):
                    t = start // P + s
                    nc.gpsimd.indirect_dma_start(
                        out=xh[:, s, :],
                        out_offset=None,
                        in_=fwd[:],
                        in_offset=IndirectOffsetOnAxis(ap=hT_sb[:, t:t + 1], axis=0),
                    )
                # pre-transposed modifier half: direct, feature-major
                xm = xm_pool.tile([P, 2, size], bf16, tag="xm", name=f"xm_{gi}")
                nc.sync.dma_start(xm[:], bwdT[:, :, start:start + size])
                xg_tiles[gi] = xh
                xm_tiles[gi] = xm

            # PE warm-up on scratch; output never read.  Queued before any
            # real matmul so the clock gate flips during the DMA prologue.
            wps = pt_pool.tile([P, 512], f32, tag="pt", name="warmup")
            for i in range(N_WARMUP):
                nc.tensor.matmul(
                    out=wps[:], lhsT=warm_sb[:, 0:P], rhs=warm_sb[:],
                    start=True, stop=True,
                )

            load_group(0)
            load_group(1)
            # identity built after the first gathers are queued: it is only
            # needed by the first PE transpose, and building it earlier
            # delays every gather on the serial gpsimd queue
            make_identity(nc, ident[:])
            # w1 split per k-chunk; modifier chunks (rows 256..511) first
            w1_sb = [const_pool.tile([P, 2 * HID], bf16, tag=f"w1_{kc}",
                                     name=f"w1_{kc}")
                     for kc in range(4)]
            for kc in (2, 3):
                nc.sync.dma_start(w1_sb[kc][:], w1[kc * P:(kc + 1) * P, :])
            load_group(2)
            for kc in (0, 1):
                nc.sync.dma_start(w1_sb[kc][:], w1[kc * P:(kc + 1) * P, :])
            w2_sb = const_pool.tile([P, 8, HID2], bf16)
            nc.sync.dma_start(w2_sb[:], w2p[:])
            load_group(3)
            w3_sb = const_pool.tile([P, 2, NREL], bf16)
            nc.sync.dma_start(w3_sb[:], w3p[:])
            load_group(4)
            load_group(5)

            xT_tiles = [None] * len(GROUPS)

            def emit_transpose(gi):
                start, size = GROUPS[gi]
                xh = xg_tiles[gi]
                xTs = []
                for kc in range(2):  # head half only
                    col = kc * P
                    pt = pt_pool.tile([P, size], bf16, tag="pt",
                                      name=f"pt_{gi}_{kc}")
                    for s in range(size // P):
                        nc.tensor.transpose(
                            pt[:, s * P:(s + 1) * P],
                            xh[:, s, col:col + P], ident[:])
                    xT = xT_pool.tile([P, size], bf16, tag="xT",
                                      name=f"xT_{gi}_{kc}")
                    nc.vector.tensor_copy(out=xT[:], in_=pt[:])
                    xTs.append(xT)
                xT_tiles[gi] = xTs

            emit_transpose(0)
            for gi, (start, size) in enumerate(GROUPS):
                xTs = xT_tiles[gi]
                xm = xm_tiles[gi]
                # ---- L1: modifier half first (kc 2,3), then head (0,1) ----
                h1s = []
                for hc in range(8):
                    ph = ph_pool.tile([P, size], f32, tag="ph",
                                      name=f"ph_{gi}_{hc}")
                    for i, kc in enumerate((2, 3, 0, 1)):
                        rhs = xm[:, kc - 2, :] if kc >= 2 else xTs[kc][:]
                        nc.tensor.matmul(
                            out=ph[:],
                            lhsT=w1_sb[kc][:, hc * P:(hc + 1) * P],
                            rhs=rhs,
                            start=(i == 0),
                            stop=(i == 3),
                        )
                    h1 = h1_pool.tile([P, size], bf16, tag="h1",
                                      name=f"h1_{gi}_{hc}")
                    nc.scalar.activation(
                        out=h1[:], in_=ph[:], func=Tanh,
                        bias=bp_sb[:, hc:hc + 1],
                    )
                    h1s.append(h1)

                # transpose the NEXT group while this group's L2/L3 run
                if gi + 1 < len(GROUPS):
                    emit_transpose(gi + 1)

                # ---- L2 ----
                h2s = []
                for jc in range(2):
                    pj = pj_pool.tile([P, size], f32, tag="pj",
                                      name=f"pj_{gi}_{jc}")
                    for kc in range(8):
                        nc.tensor.matmul(
                            out=pj[:],
                            lhsT=w2_sb[:, kc, jc * P:(jc + 1) * P],
                            rhs=h1s[kc][:],
                            start=(kc == 0),
                            stop=(kc == 7),
                        )
                    h2 = h2_pool.tile([P, size], bf16, tag="h2",
                                      name=f"h2_{gi}_{jc}")
                    nc.scalar.activation(
                        out=h2[:], in_=pj[:], func=Tanh,
                        bias=bp_sb[:, 8 + jc:9 + jc],
                    )
                    h2s.append(h2)

                # ---- L3 (b3 on host) ----
                po = pt_pool.tile([NREL, size], f32, tag="pt", name=f"po_{gi}")
                for kc in range(2):
                    nc.tensor.matmul(
                        out=po[:],
                        lhsT=w3_sb[:, kc, :],
                        rhs=h2s[kc][:],
                        start=(kc == 0),
                        stop=(kc == 1),
                    )
                o = out_pool.tile([NREL, size], f32, tag="o", name=f"o_{gi}")
                nc.vector.tensor_copy(out=o[:], in_=po[:])
                nc.sync.dma_start(outT[:, start:start + size], o[:])

    nc.finalize()
    return nc


def _prep_inputs(inputs, rhidLayerFOH, rhidLayerFOM, rcatBias, rhid2Layer,
                 rhid2Bias, routLayer, routBias, heads):
    wdt = ml_dtypes.bfloat16
    inputs = np.asarray(inputs, dtype=np.float32)
    heads = np.asarray(heads)

    fwd = np.ascontiguousarray(inputs[:, 0, :]).astype(wdt)
    bwd_full = inputs[:, 1, :]
    mods_pad = np.concatenate(
        [np.arange(1, N_TOKENS), [N_TOKENS - 1]]).astype(np.int64)
    heads_pad = np.concatenate([heads.astype(np.int64), [0]]).astype(np.int32)

    w1 = np.ascontiguousarray(
        np.concatenate([np.asarray(rhidLayerFOH), np.asarray(rhidLayerFOM)],
                       axis=1)).astype(wdt)                      # [512, 1024]
    w2p = np.ascontiguousarray(
        np.asarray(rhid2Layer, dtype=np.float32)
        .reshape(8, P, HID2).transpose(1, 0, 2)).astype(wdt)
    w3p = np.ascontiguousarray(
        np.asarray(routLayer, dtype=np.float32)
        .reshape(2, P, NREL).transpose(1, 0, 2)).astype(wdt)
    b1 = np.asarray(rcatBias, dtype=np.float32).reshape(8, P).T
    b2 = np.asarray(rhid2Bias, dtype=np.float32).reshape(2, P).T
    bpack = np.ascontiguousarray(np.concatenate([b1, b2], axis=1))

    in_maps = []
    for c in range(NCORES):
        sl = slice(c * EPC, (c + 1) * EPC)
        bwd_c = bwd_full[mods_pad[sl]]                           # [2048, 256]
        bwdT_c = np.ascontiguousarray(
            bwd_c.T.reshape(2, P, EPC).transpose(1, 0, 2)).astype(wdt)
        headsT_c = np.ascontiguousarray(heads_pad[sl].reshape(SUB, P).T)
        in_maps.append({
            "fwd": fwd, "bwdT": bwdT_c, "headsT": headsT_c,
            "w1": w1, "w2p": w2p, "w3p": w3p, "bpack": bpack,
        })
    return in_maps


def kernel(inputs, rhidLayerFOH, rhidLayerFOM, rcatBias, rhid2Layer, rhid2Bias,
           routLayer, routBias, heads):
    global LAST_RESULTS

    in_maps = _prep_inputs(inputs, rhidLayerFOH, rhidLayerFOM, rcatBias,
                           rhid2Layer, rhid2Bias, routLayer, routBias, heads)

    if "nc" not in _CACHE:
        _CACHE["nc"] = _build()
    nc = _CACHE["nc"]

    trace_dir = os.environ.get("KERNEL_TRACE_DIR") or None
    res = run_bass_kernel_spmd(nc, in_maps, list(range(NCORES)), tmpdir=trace_dir)
    LAST_RESULTS = res

    outT = np.concatenate([r["outT"] for r in res.results], axis=1)
    out = outT.T[:E] + np.asarray(routBias, dtype=np.float32)[None, :]
    return np.ascontiguousarray(out).astype(np.float32)
